# revision 1
# baseline (speedup 1.0000x reference)
"""Differential attention kernel for Trainium2, 8-core SPMD.

Problem: B=2, S=2048, D=1024, 16 heads x 64 head-dim differential attention
(two softmaxes, combined with a scalar lambda), with input/output projections.

Sharding: data-parallel over batch (2 groups of 4 cores) x tensor-parallel
over heads (4 heads per core). Each core computes q/k/v projections for its
4 heads, both attention softmaxes, and a partial output projection
(its heads' rows of Wo). Host sums the 4 partial outputs per batch, adds bo.

On-chip layout is "feature x token" (transposed) throughout:
  - hsT [D, S] streams as the moving operand of the projections.
  - qT/kT [128, S] per head, rows 0-63 = q1/k1 head-dim, rows 64-127 = q2/k2.
    The two score matmuls (K=64 each) are issued to PE row groups (0,0) and
    (64,0) and run concurrently in the array.
  - Scores are computed transposed, sT[k, q], so softmax sums land in the PV
    matmul via an appended ones-column on v (no partition-dim reduction).
  - exp() runs on the scalar engine straight out of PSUM with the attention
    mask folded into the per-partition bias and the 1/sqrt(hd) scale folded
    into the activation scale.
  - Normalization: reciprocal of the ones-column sums (exact DVE reciprocal;
    the custom-DVE approx op returns garbage through this runtime), bounced
    through a DRAM scratch row and broadcast-loaded across 64 partitions by a
    step-0-partition DMA, then multiply/multiply/subtract with lambda folded
    into one row scale.
All matmuls run in bf16 with fp32 PSUM accumulation; output partials ship as
fp16 and are reduced across cores in fp32 on the host.

Measured: rel err vs the fp32 jax reference 6.0e-3; per-core time ~354 us by
the TimelineSim cost model (~333 us with PE row-group concurrency modeled),
consistent with hardware repeat-slope measurements (~290-480 us, axon-tunnel
noise limited).
"""

import sys

sys.path.insert(0, "/opt/trn_rl_repo")

from contextlib import ExitStack

import ml_dtypes
import numpy as np

import concourse.bacc as bacc
import concourse.tile as tile
from concourse import mybir
from concourse.bass_utils import run_bass_kernel_spmd

B, S, D = 2, 2048, 1024
NH, HD = 16, 64
NCORES = 8
HPC = 4              # heads per core
QB = 512             # q block (free dim of score matmuls)
NJ = S // QB         # 4
KC = 128             # k chunk (partition dim of transposed scores)
NKC = S // KC        # 16
NDI = D // 128       # 8 contraction chunks for projections
VA = HD + 1          # v columns per head incl. ones column

BF16 = mybir.dt.bfloat16
F32 = mybir.dt.float32
npbf16 = ml_dtypes.bfloat16

# Module-level cache: the Bass module depends only on shapes and lambda.
_BUILD_CACHE = {}
TRACE = False
LAST_RESULTS = None


RECIP_APPROX = False  # custom-DVE approx reciprocal is broken on HW via this path
ROW_PAIR = True       # issue the two K=64 score matmuls to PE row groups 0/64


def _build(lam: float, with_bias: bool = True, repeat: int = 1):
    nc = bacc.Bacc(None, target_bir_lowering=False)

    hst_d = nc.dram_tensor("hst", [D, S], BF16, kind="ExternalInput")
    wq_d = nc.dram_tensor("wq", [D, 2 * HPC * HD], BF16, kind="ExternalInput")
    wk_d = nc.dram_tensor("wk", [D, 2 * HPC * HD], BF16, kind="ExternalInput")
    wv_d = nc.dram_tensor("wv", [D, HPC * HD], BF16, kind="ExternalInput")
    wo_d = nc.dram_tensor("wo", [HPC * HD, D], BF16, kind="ExternalInput")
    bq_d = nc.dram_tensor("bq", [1, 2 * HPC * HD], BF16, kind="ExternalInput")
    bk_d = nc.dram_tensor("bk", [1, 2 * HPC * HD], BF16, kind="ExternalInput")
    bv_d = nc.dram_tensor("bv", [1, HPC * HD], BF16, kind="ExternalInput")
    mask_d = nc.dram_tensor("maskc", [KC, NKC], F32, kind="ExternalInput")
    out_d = nc.dram_tensor("outT", [D, S], mybir.dt.float16, kind="ExternalOutput")

    with tile.TileContext(nc) as tc, ExitStack() as ctx:
        const = ctx.enter_context(tc.tile_pool(name="const", bufs=1))
        wpool = ctx.enter_context(tc.tile_pool(name="wpool", bufs=1))
        hpool = ctx.enter_context(tc.tile_pool(name="hpool", bufs=1))
        qkpool = ctx.enter_context(tc.tile_pool(name="qkpool", bufs=1))
        vpool = ctx.enter_context(tc.tile_pool(name="vpool", bufs=1))
        epool = ctx.enter_context(tc.tile_pool(name="epool", bufs=24))
        rpool = ctx.enter_context(tc.tile_pool(name="rpool", bufs=3))
        bpool = ctx.enter_context(tc.tile_pool(name="bpool", bufs=3))
        tpool = ctx.enter_context(tc.tile_pool(name="tpool", bufs=4))
        spool = ctx.enter_context(tc.tile_pool(name="spool", bufs=16))
        opool = ctx.enter_context(tc.tile_pool(name="opool", bufs=4))
        dpool = ctx.enter_context(tc.tile_pool(name="dpool", bufs=4, space="DRAM"))
        ps_sc = ctx.enter_context(tc.tile_pool(name="ps_sc", bufs=2, space="PSUM"))
        ps_pv = ctx.enter_context(tc.tile_pool(name="ps_pv", bufs=2, space="PSUM"))
        ps_mm = ctx.enter_context(tc.tile_pool(name="ps_mm", bufs=2, space="PSUM"))

        # ---- constants and weights in SBUF (per-chunk DMAs so the first
        # projection matmuls start as soon as their operands land) ----
        def alloc_chunked(pool, width, tagp):
            return [pool.tile([128, width], BF16, tag=f"{tagp}{c}", name=f"{tagp}{c}")
                    for c in range(NDI)]

        wqt = alloc_chunked(wpool, 2 * HPC * HD, "wq")
        wkt = alloc_chunked(wpool, 2 * HPC * HD, "wk")
        wvt = alloc_chunked(wpool, HPC * HD, "wv")
        hst = [[hpool.tile([128, QB], BF16, tag=f"hs{c}_{j}", name=f"hs{c}_{j}")
                for j in range(NJ)] for c in range(NDI)]
        for c in range(NDI):
            nc.scalar.dma_start(
                out=hst[c][0][:], in_=hst_d[c * 128:(c + 1) * 128, 0:QB]
            )
            nc.sync.dma_start(out=wqt[c][:], in_=wq_d[c * 128:(c + 1) * 128, :])
            nc.gpsimd.dma_start(out=wkt[c][:], in_=wk_d[c * 128:(c + 1) * 128, :])
        for j in range(1, NJ):
            for c in range(NDI):
                nc.scalar.dma_start(
                    out=hst[c][j][:],
                    in_=hst_d[c * 128:(c + 1) * 128, j * QB:(j + 1) * QB],
                )
        for c in range(NDI):
            nc.gpsimd.dma_start(out=wvt[c][:], in_=wv_d[c * 128:(c + 1) * 128, :])
        wot = []
        for h in range(HPC):
            t = wpool.tile([HD, D], BF16, tag=f"wo{h}")
            nc.gpsimd.dma_start(out=t[:], in_=wo_d[h * HD:(h + 1) * HD, :])
            wot.append(t)
        bqt = const.tile([1, 2 * HPC * HD], BF16, tag="bq")
        nc.gpsimd.dma_start(out=bqt[:], in_=bq_d[:])
        bkt = const.tile([1, 2 * HPC * HD], BF16, tag="bk")
        nc.gpsimd.dma_start(out=bkt[:], in_=bk_d[:])
        bvt = const.tile([1, HPC * HD], BF16, tag="bv")
        nc.gpsimd.dma_start(out=bvt[:], in_=bv_d[:])
        maskt = const.tile([KC, NKC], F32, tag="mask")
        nc.gpsimd.dma_start(out=maskt[:], in_=mask_d[:])
        ones = const.tile([1, S], BF16, tag="ones")
        nc.gpsimd.memset(ones[:], 1.0)

        # persistent per-head qT/kT ([q1;q2] stacked on partitions) and v_aug
        QP = 128 if ROW_PAIR else 64

        def emit_qk_proj(h):
            # qT/kT for head h: psum [QP, QB] accumulated over 8 di-chunks
            # plus a K=1 bias matmul, evicted (cast) to bf16.
            nhalf = 128 // QP
            for j in range(NJ):
                for wt, bt, dsts in ((wqt, bqt, qt), (wkt, bkt, kt)):
                    for half in range(nhalf):
                        dst = dsts[h * nhalf + half]
                        lo = h * 128 + half * QP
                        ps = ps_mm.tile([QP, QB], F32, tag="mm")
                        for c in range(NDI):
                            nc.tensor.matmul(
                                ps[:],
                                lhsT=wt[c][:, lo:lo + QP],
                                rhs=hst[c][j][:],
                                start=(c == 0),
                                stop=(not with_bias and c == NDI - 1),
                            )
                        if with_bias:
                            nc.tensor.matmul(
                                ps[:],
                                lhsT=bt[0:1, lo:lo + QP],
                                rhs=ones[0:1, j * QB:(j + 1) * QB],
                                start=False,
                                stop=True,
                            )
                        nc.vector.tensor_copy(dst[j][:], ps[:])

        def emit_v_proj():
            # v[s, 4*64] per s-chunk, scattered into v_aug (65-wide head blocks,
            # ones column preset by memset).
            for sc in range(NKC):
                ps = ps_mm.tile([128, HPC * HD], F32, tag="mm")
                for c in range(NDI):
                    nc.tensor.matmul(
                        ps[:],
                        lhsT=hst[c][sc // 4][:, (sc % 4) * 128:(sc % 4 + 1) * 128],
                        rhs=wvt[c][:],
                        start=(c == 0),
                        stop=(not with_bias and c == NDI - 1),
                    )
                if with_bias:
                    nc.tensor.matmul(
                        ps[:],
                        lhsT=ones[0:1, 0:128],
                        rhs=bvt[0:1, :],
                        start=False,
                        stop=True,
                    )
                src = ps[:].rearrange("p (h x) -> p h x", x=HD)
                dst = va[sc][:].rearrange("p (h y) -> p h y", y=VA)[:, :, 0:HD]
                nc.vector.tensor_copy(dst, src)

        def emit_score_exp(j, h, c):
            # transposed scores for q-block j, head h, k-chunk c; both halves.
            sp = ps_sc.tile([128, 2 * QB], F32, tag="sp")
            kj, kcol = divmod(c * KC, QB)
            if ROW_PAIR:
                score_ops = (
                    (kt[h][kj][0:64, :], qt[h][j][0:64, :], 0),
                    (kt[h][kj][64:128, :], qt[h][j][64:128, :], QB),
                )
            else:
                score_ops = (
                    (kt[2 * h][kj][:], qt[2 * h][j][:], 0),
                    (kt[2 * h + 1][kj][:], qt[2 * h + 1][j][:], QB),
                )
            for ksrc, qsrc, off in score_ops:
                nc.tensor.matmul(
                    sp[:, off:off + QB],
                    lhsT=ksrc[:, kcol:kcol + KC],
                    rhs=qsrc[:],
                    start=True,
                    stop=True,
                )
            et = epool.tile([128, 2 * QB], BF16, tag="et")
            nc.scalar.activation(
                et[:],
                sp[:],
                mybir.ActivationFunctionType.Exp,
                bias=maskt[:, c:c + 1],
                scale=float(HD) ** -0.5,
            )
            return et

        def emit_pv(h, c, et, pv1, pv2):
            nc.tensor.matmul(
                pv1[:],
                lhsT=va[c][:, h * VA:(h + 1) * VA],
                rhs=et[:, 0:QB],
                start=(c == 0),
                stop=(c == NKC - 1),
            )
            nc.tensor.matmul(
                pv2[:],
                lhsT=va[c][:, h * VA:(h + 1) * VA],
                rhs=et[:, QB:2 * QB],
                start=(c == 0),
                stop=(c == NKC - 1),
            )

        def emit_attn(j, h, stg, ets=None):
            pv1 = ps_pv.tile([VA, QB], F32, tag="pv")
            pv2 = ps_pv.tile([VA, QB], F32, tag="pv")
            for c in range(NKC):
                if ets is not None and c < len(ets):
                    et = ets[c]
                else:
                    et = emit_score_exp(j, h, c)
                emit_pv(h, c, et, pv1, pv2)
            # evict PV accumulators to SBUF promptly so the PSUM slots recycle
            pc1 = tpool.tile([VA, QB], F32, tag="pc")
            pc2 = tpool.tile([VA, QB], F32, tag="pc")
            nc.vector.tensor_copy(pc1[:], pv1[:])
            nc.vector.tensor_copy(pc2[:], pv2[:])
            # normalize + combine: out = pv1/r1 - lam * pv2/r2
            r1 = rpool.tile([VA, QB], F32, tag="r")
            r2 = rpool.tile([VA, QB], F32, tag="r")
            if RECIP_APPROX:
                nc.vector.reciprocal_approx_fast(out=r1[HD:VA, :], in_=pc1[HD:VA, :])
                nc.vector.reciprocal_approx_fast(out=r2[HD:VA, :], in_=pc2[HD:VA, :])
            else:
                nc.vector.reciprocal(out=r1[HD:VA, :], in_=pc1[HD:VA, :])
                nc.vector.reciprocal(out=r2[HD:VA, :], in_=pc2[HD:VA, :])
            nc.vector.tensor_scalar_mul(r2[HD:VA, :], r2[HD:VA, :], float(lam))
            # SBUF has no partition-broadcast path: bounce each recip row
            # through DRAM, then broadcast-load (step-0 partition dim).
            rb1 = dpool.tile([1, QB], F32, tag="rb", name="rb1")
            rb2 = dpool.tile([1, QB], F32, tag="rb", name="rb2")
            nc.sync.dma_start(out=rb1[:], in_=r1[HD:VA, :])
            nc.sync.dma_start(out=rb2[:], in_=r2[HD:VA, :])
            bc1 = bpool.tile([HD, QB], F32, tag="bc")
            bc2 = bpool.tile([HD, QB], F32, tag="bc")
            nc.sync.dma_start(out=bc1[:], in_=rb1[:].to_broadcast([HD, QB]))
            nc.sync.dma_start(out=bc2[:], in_=rb2[:].to_broadcast([HD, QB]))
            t1 = tpool.tile([HD, QB], F32, tag="tmp")
            t2 = tpool.tile([HD, QB], F32, tag="tmp")
            nc.vector.tensor_mul(t1[:], pc1[0:HD, :], bc1[:])
            nc.vector.tensor_mul(t2[:], pc2[0:HD, :], bc2[:])
            nc.vector.tensor_sub(stg[:], t1[:], t2[:])

        def emit_outproj(j, stgs):
            # partial out-projection: outT[do, qblock] = sum_h wo_h.T @ stg_h.
            # Evictions alternate DVE/ACT (ACT is idle in the tail); partials
            # ship as fp16 — the host reduces across cores in fp32.
            for d in range(NDI):
                ps = ps_mm.tile([128, QB], F32, tag="mm")
                for h in range(HPC):
                    nc.tensor.matmul(
                        ps[:],
                        lhsT=wot[h][:, d * 128:(d + 1) * 128],
                        rhs=stgs[h][:],
                        start=(h == 0),
                        stop=(h == HPC - 1),
                    )
                ot = opool.tile([128, QB], mybir.dt.float16, tag="ot")
                if d % 2 == 0:
                    nc.vector.tensor_copy(ot[:], ps[:])
                else:
                    nc.scalar.copy(ot[:], ps[:])
                nc.sync.dma_start(
                    out=out_d[d * 128:(d + 1) * 128, j * QB:(j + 1) * QB],
                    in_=ot[:],
                )

        # ---- emission order: heads outer so ACT streams without gaps;
        # head h+1's projections fill PE slack during head h's attention;
        # out-projection for block j trails once the last head's stage lands.
        for _rep in range(repeat):
            qt = [[qkpool.tile([QP, QB], BF16, tag=f"qt{h}_{j}", name=f"qt{h}_{j}")
                   for j in range(NJ)] for h in range(HPC * (128 // QP))]
            kt = [[qkpool.tile([QP, QB], BF16, tag=f"kt{h}_{j}", name=f"kt{h}_{j}")
                   for j in range(NJ)] for h in range(HPC * (128 // QP))]
            va = [vpool.tile([128, HPC * VA], BF16, tag=f"va{c}", name=f"va{c}")
                  for c in range(NKC)]
            for c in range(NKC):
                nc.gpsimd.memset(va[c][:], 1.0)
            emit_qk_proj(0)
            # scores/exp of (h0, j0) outrank the v-projection so ACT starts
            # early; their PV consumers are emitted after v_aug is produced.
            ets00 = [emit_score_exp(0, 0, c) for c in range(NKC)]
            ets01 = [emit_score_exp(1, 0, c) for c in range(NKC // 2)]
            emit_v_proj()
            stgs = [[spool.tile([HD, QB], BF16, tag="stg", name=f"stg{j}_{i}")
                     for i in range(HPC)] for j in range(NJ)]
            for h in range(HPC):
                for j in range(NJ):
                    pre = {(0, 0): ets00, (0, 1): ets01}.get((h, j))
                    emit_attn(j, h, stgs[j][h], ets=pre)
                    if h < HPC - 1 and j == NJ - 1:
                        emit_qk_proj(h + 1)
            for j in range(NJ):
                emit_outproj(j, stgs[j])

    nc.compile()
    return nc


def _prep_inputs(hidden_states, attention_mask, Wq, bq, Wk, bk, Wv, bv, Wo):
    """Build the 8 per-core input maps (host-side shard + transpose + cast)."""
    in_maps = []
    hsT = [np.ascontiguousarray(hidden_states[b].T).astype(npbf16) for b in range(B)]
    maskc = [
        np.ascontiguousarray(
            ((1.0 - attention_mask[b]) * -10000.0).astype(np.float32).reshape(NKC, KC).T
        )
        for b in range(B)
    ]
    for core in range(NCORES):
        b = core // (NCORES // B)
        hb = (core % (NCORES // B)) * HPC
        heads = range(hb, hb + HPC)
        qk_idx = np.concatenate(
            [np.r_[h * HD:(h + 1) * HD, D + h * HD:D + (h + 1) * HD] for h in heads]
        )
        v_idx = np.r_[hb * HD:(hb + HPC) * HD]
        in_maps.append(
            {
                "hst": hsT[b],
                "wq": np.ascontiguousarray(Wq[:, qk_idx]).astype(npbf16),
                "wk": np.ascontiguousarray(Wk[:, qk_idx]).astype(npbf16),
                "wv": np.ascontiguousarray(Wv[:, v_idx]).astype(npbf16),
                "wo": np.ascontiguousarray(Wo[v_idx, :]).astype(npbf16),
                "bq": bq[qk_idx].reshape(1, -1).astype(npbf16),
                "bk": bk[qk_idx].reshape(1, -1).astype(npbf16),
                "bv": bv[v_idx].reshape(1, -1).astype(npbf16),
                "maskc": maskc[b],
            }
        )
    return in_maps


def kernel(
    hidden_states,
    attention_mask,
    Wq,
    bq,
    Wk,
    bk,
    Wv,
    bv,
    Wo,
    bo,
    lq1,
    lk1,
    lq2,
    lk2,
):
    global LAST_RESULTS
    args = [hidden_states, attention_mask, Wq, bq, Wk, bk, Wv, bv, Wo, bo]
    hidden_states, attention_mask, Wq, bq, Wk, bk, Wv, bv, Wo, bo = (
        np.asarray(a, dtype=np.float32) for a in args
    )
    lq1, lk1, lq2, lk2 = (np.asarray(a, dtype=np.float64) for a in (lq1, lk1, lq2, lk2))
    lam = float(np.exp(lq1 @ lk1) - np.exp(lq2 @ lk2) + 0.8)

    with_bias = not (
        np.all(bq == 0) and np.all(bk == 0) and np.all(bv == 0)
    )
    key = (round(lam, 9), with_bias)
    if key not in _BUILD_CACHE:
        _BUILD_CACHE.clear()
        _BUILD_CACHE[key] = _build(lam, with_bias)
    nc = _BUILD_CACHE[key]

    in_maps = _prep_inputs(hidden_states, attention_mask, Wq, bq, Wk, bk, Wv, bv, Wo)
    res = run_bass_kernel_spmd(nc, in_maps, core_ids=list(range(NCORES)), trace=TRACE)
    LAST_RESULTS = res

    out = np.empty((B, S, D), dtype=np.float32)
    gpb = NCORES // B
    for b in range(B):
        acc = res.results[b * gpb]["outT"].astype(np.float32)
        for g in range(1, gpb):
            acc = acc + res.results[b * gpb + g]["outT"]
        out[b] = acc.T + bo[None, :]
    return out

